# revision 5
# baseline (speedup 1.0000x reference)
"""Trainium2 Bass kernel for nn_CP2_17669495456475 (dynamic-kernel deconv).

Math: out[b,c,y,x] = sum_l cos[b,l,i,j] * W[b,l,c,ky,kx],  y=8i+ky, x=8j+kx,
with W = unfold(pad(b)) * (1 - unfold(pad(mask))), K=16, S=8, crop 4.

Reformulation: the 4-fold (sy,sx) deconv overlap sum is pre-folded into the
MOVING operand, so the main matmul contracts over only the 33x33=1089 bm
blocks (3.76x fewer PE cycles than the 4096-contraction chunk scheme):

  A[(q,r),(u,v)] = sum_{sy,sx in {0,1}} cos[q-sy, r-sx, u-sy, v-sx]
  out[(c,ry,rx),(u,v)] = sum_{(q,r)} bm_block[(q,r),(c,ry,rx)] * A[(q,r),(u,v)]

On-device A build, per 128-row contraction tile (rows flat r-major
F = 33r+q, tiles at stride 127 so the sy fold's F-1 shift stays IN-tile):
  - one HBM read t01 = two j/row-offset copies of zero-padded cos
    (the sx fold's two terms, host-interleaved so one DMA fetches both)
  - b1 = t01[pair0] + t01[pair1]                (DVE add: sx fold)
  - sh2 = b1[1:128] shifted down one partition  (HWDGE SBUF->SBUF copy)
  - at = sh2 + b1[0:127]                        (DVE add: sy fold)
The mask multiply (1-m)*b stays on-device (GpSimd).

Sharding: 8 cores = 4 batches x 2 channel-halves (16 ch each). Full inputs
in, full output out; host does layout/pad glue only.
"""

import numpy as np

import concourse.bass as bass
import concourse.mybir as mybir
import concourse.tile as tile
from concourse.bass_utils import run_bass_kernel_spmd

PD = 4
C = 16              # channels per core
N_CORES = 8
NV = 34             # v columns per u row (33 real + 1 zero pad)
NT = 11 * NV        # N per matmul: 11 u-rows x NV v-cols
NK = 9              # contraction tiles (127-row stride, 128-row reads)


def _split_multi_sync(nc):
    """The walrus in this env allows only ONE sync-wait per instruction.
    Hoist extra waits onto same-engine InstNoOp carriers placed just before
    the owning instruction (sequential waits on one engine == AND)."""
    ctr = 0
    for f in nc.m.functions:
        for bb in f.blocks:
            insts = list(bb.instructions)
            out = []
            changed = False
            for inst in insts:
                si = inst.sync_info
                waits = list(si.on_wait) if si and si.on_wait else []
                if len(waits) > 1:
                    for w in waits[:-1]:
                        nop = mybir.InstNoOp(name=f"waitnop-{ctr}", ins=[], outs=[])
                        ctr += 1
                        nop.engine = inst.engine
                        nop.sync_info = mybir.SyncInfo(on_wait=[w], on_update=[])
                        out.append(nop)
                    si.on_wait = [waits[-1]]
                    changed = True
                out.append(inst)
            if changed:
                bb.instructions = out
    return ctr


def _build_nc():
    f32 = mybir.dt.float32
    bf16 = mybir.dt.bfloat16
    nc = bass.Bass(enable_partition_id=False)
    # interleaved zero-padded cos pair (r-major rows F=33r+q):
    #   cdX[R, 0, 1+i, 2+j] = cosR[R-1][i,j]   (sx=0 term)
    #   cdX[R, 1, 1+i, 3+j] = cosR[R-34][i,j]  (sx=1 term: row F-33, j v-1)
    cdX = nc.declare_dram_parameter("cdX", [1144, 2, 34, 36], bf16, isOutput=False)
    # mask+bm blocks, row-duplicated per 127-stride tile: wbmD[k, p] = row 127k+p,
    # cols [0:64) = m block, [64:1088) = b block
    wbmD = nc.declare_dram_parameter("wbmD", [NK, 128, 1088], bf16, isOutput=False)
    # out: m-major [m, p, n, NT] so each m writes one DMA
    outT = nc.declare_dram_parameter("outT", [8, 128, 3, NT], bf16, isOutput=True)

    with tile.TileContext(nc) as tc:
        with (
            tc.tile_pool(name="cp", bufs=3) as cp,
            tc.tile_pool(name="b1p", bufs=3) as b1p,
            tc.tile_pool(name="shp", bufs=3) as shp,
            tc.tile_pool(name="atp", bufs=NK) as atp,
            tc.tile_pool(name="wbp", bufs=3) as wbp,
            tc.tile_pool(name="msp", bufs=2) as msp,
            tc.tile_pool(name="wmp", bufs=NK) as wmp,
            tc.tile_pool(name="op", bufs=4) as op,
            tc.tile_pool(name="pp", bufs=6, space="PSUM") as pp,
        ):
            # --- input DMAs, priority-ordered on the sync queue ---
            t01s, wb3s = [None] * NK, [None] * 3
            wb3s[0] = wbp.tile([128, 3, 1088], bf16, tag="wb", name="wb3_0")
            nc.sync.dma_start(wb3s[0][:], wbmD[0:3].rearrange("k p f -> p k f"))
            for k in range(NK):
                t01 = cp.tile([128, 2, 34, 36], bf16, tag="t01", name=f"t01_{k}")
                nc.sync.dma_start(t01[:], cdX[127 * k: 127 * k + 128])
                t01s[k] = t01
                if k == 2:
                    wb3s[1] = wbp.tile([128, 3, 1088], bf16, tag="wb", name="wb3_1")
                    nc.sync.dma_start(wb3s[1][:],
                                      wbmD[3:6].rearrange("k p f -> p k f"))
                if k == 5:
                    wb3s[2] = wbp.tile([128, 3, 1088], bf16, tag="wb", name="wb3_2")
                    nc.sync.dma_start(wb3s[2][:],
                                      wbmD[6:9].rearrange("k p f -> p k f"))

            # --- mask multiply on GpSimd (otherwise idle) ---
            wmks = []
            for k in range(NK):
                g, j = k // 3, k % 3
                ms = msp.tile([128, 64], bf16, tag="ms", name=f"ms_{k}")
                nc.gpsimd.tensor_scalar(
                    out=ms[:], in0=wb3s[g][:, j, 0:64], scalar1=-1.0, scalar2=1.0,
                    op0=mybir.AluOpType.mult, op1=mybir.AluOpType.add,
                )
                wmk = wmp.tile([128, C * 64], bf16, tag="wm", name=f"wm_{k}")
                nc.gpsimd.tensor_tensor(
                    out=wmk[:].rearrange("p (c f) -> p c f", c=C),
                    in0=wb3s[g][:, j, 64:1088].rearrange("p (c f) -> p c f", c=C),
                    in1=ms[:, None, :].broadcast_to([128, C, 64]),
                    op=mybir.AluOpType.mult,
                )
                wmks.append(wmk)

            # --- A build: sx fold (DVE), sy fold (shift-copy + DVE) ---
            # vector-queue order staggers at_k two slots behind b1_{k} so the
            # sh2 DMA round trip never head-of-line-blocks the queue.
            ats, b1s, sh2s = [], [], []

            def emit_b1(k):
                b1 = b1p.tile([128, 34, 34], bf16, tag="b1", name=f"b1_{k}")
                nc.vector.tensor_tensor(
                    out=b1[:], in0=t01s[k][:, 0, :, 2:36],
                    in1=t01s[k][:, 1, :, 2:36], op=mybir.AluOpType.add,
                )
                b1s.append(b1)
                sh2 = shp.tile([128, 33, NV], bf16, tag="sh", name=f"sh2_{k}")
                nc.scalar.dma_start(sh2[0:127], b1[1:128, 1:34, :])
                sh2s.append(sh2)

            def emit_at(k):
                at = atp.tile([128, 33, NV], bf16, tag="at", name=f"at_{k}")
                nc.vector.tensor_tensor(
                    out=at[:], in0=sh2s[k][:], in1=b1s[k][:, 0:33, :],
                    op=mybir.AluOpType.add,
                )
                ats.append(at)

            for k in range(NK):
                emit_b1(k)
                if k >= 2:
                    emit_at(k - 2)
            emit_at(NK - 2)
            emit_at(NK - 1)

            # --- main matmul: m-outer, k-contiguous, 3 n-phases ---
            # psum tiles padded to a full 2KB bank (no bank sharing).
            for m in range(8):
                pss = [pp.tile([128, 512], f32, tag="ps", name=f"ps_{m}_{n}")
                       for n in range(3)]
                for k in range(NK):
                    lhs = wmks[k][0:127, 128 * m: 128 * (m + 1)]
                    for n in range(3):
                        nc.tensor.matmul(
                            pss[n][:, 0:NT], lhs,
                            ats[k][0:127, 11 * n: 11 * n + 11, :],
                            start=(k == 0), stop=(k == NK - 1),
                        )
                ot = op.tile([128, 3, NT], bf16, tag="o", name=f"osb_{m}")
                nc.scalar.copy(ot[:, 0, :], pss[0][:, 0:NT])
                nc.vector.tensor_copy(ot[:, 1, :], pss[1][:, 0:NT])
                nc.scalar.copy(ot[:, 2, :], pss[2][:, 0:NT])
                nc.scalar.dma_start(outT[m], ot[:])

    _split_multi_sync(nc)
    return nc


def _host_prep(b_ch, mask_b, cos_b):
    """b_ch (16,256,256) f32, mask_b (256,256) f32, cos_b (1024,32,32) f32
    -> dict of device inputs (layout/pad glue only)."""
    import ml_dtypes
    bf16 = ml_dtypes.bfloat16
    bpad = np.pad(b_ch, ((0, 0), (PD, PD), (PD, PD)), mode="edge")
    mpad = np.pad(mask_b, ((PD, PD), (PD, PD)), mode="edge")

    cosR = np.zeros((1089, 32, 32), bf16)              # r-major rows F=33r+q
    cosR.reshape(33, 33, 32, 32)[:32, :32] = \
        cos_b.reshape(32, 32, 32, 32).transpose(1, 0, 2, 3).astype(bf16)
    cdX = np.zeros((1144, 2, 34, 36), bf16)
    cdX[1:1090, 0, 1:33, 2:34] = cosR
    cdX[34:1123, 1, 1:33, 3:35] = cosR

    wbm = np.zeros((1144, 1088), bf16)
    wbm[:1089, 0:64] = mpad.reshape(33, 8, 33, 8).transpose(2, 0, 1, 3) \
        .reshape(1089, 64).astype(bf16)
    wbm[:1089, 64:] = bpad.reshape(C, 33, 8, 33, 8).transpose(3, 1, 0, 2, 4) \
        .reshape(1089, C * 64).astype(bf16)
    wbmD = np.stack([wbm[127 * k: 127 * k + 128] for k in range(NK)])
    return {"cdX": cdX, "wbmD": wbmD}


def _unshard(outT):
    # outT [8, 128, 3, NT] -> [(c,ry,rx)=128m+p, u=11n+u', v] -> (16,256,256)
    t = np.asarray(outT, dtype=np.float32).reshape(1024, 33, NV)[:, :, :33]
    t = t.reshape(C, 8, 8, 33, 33).transpose(0, 3, 1, 4, 2)
    return t.reshape(C, 264, 264)[:, PD:260, PD:260]


_RUN_KW = {}   # test harness may inject e.g. trace=True
_LAST_RESULTS = [None]
_NC_CACHE = {}


def _get_nc():
    nc = _NC_CACHE.get("v5")
    if nc is None:
        nc = _NC_CACHE["v5"] = _build_nc()
    return nc


def kernel(cos_similar, b, mask):
    cos_similar = np.ascontiguousarray(np.asarray(cos_similar, dtype=np.float32))
    b = np.ascontiguousarray(np.asarray(b, dtype=np.float32))
    mask = np.ascontiguousarray(np.asarray(mask, dtype=np.float32))

    in_maps = []
    for core in range(N_CORES):
        batch, half = core // 2, core % 2
        ch0 = C * half
        in_maps.append(_host_prep(
            b[batch, ch0:ch0 + C], mask[batch, 0], cos_similar[batch]))

    nc = _get_nc()
    res = run_bass_kernel_spmd(nc, in_maps, list(range(N_CORES)), **_RUN_KW)
    _LAST_RESULTS[0] = res

    out = np.empty((4, 32, 256, 256), np.float32)
    for core in range(N_CORES):
        batch, half = core // 2, core % 2
        ch0 = C * half
        out[batch, ch0:ch0 + C] = _unshard(res.results[core]["outT"])
    return out


# revision 7
# speedup vs baseline: 1.1421x; 1.1421x over previous
"""Trainium2 Bass kernel for nn_CP2_17669495456475 (dynamic-kernel deconv).

Math: out[b,c,y,x] = sum_l cos[b,l,i,j] * W[b,l,c,ky,kx],  y=8i+ky, x=8j+kx,
with W = unfold(pad(b)) * (1 - unfold(pad(mask))), K=16, S=8, crop 4.

Reformulation: the 4-fold (sy,sx) deconv overlap sum is pre-folded into the
MOVING operand, so the main matmul contracts over only the 33x33=1089 bm
blocks (3.76x fewer PE cycles than the 4096-contraction chunk scheme):

  A[(q,r),(u,v)] = sum_{sy,sx in {0,1}} cos[q-sy, r-sx, u-sy, v-sx]
  out[(c,ry,rx),(u,v)] = sum_{(q,r)} bm_block[(q,r),(c,ry,rx)] * A[(q,r),(u,v)]

On-device A build, per 128-row contraction tile (rows flat r-major
F = 33r+q, tiles at stride 127 so the sy fold's F-1 shift stays IN-tile):
  - one HBM read t01 = two j/row-offset copies of zero-padded cos
    (the sx fold's two terms, host-interleaved so one DMA fetches both)
  - b1 = t01[pair0] + t01[pair1]                (DVE add: sx fold)
  - sh2 = b1[1:128] shifted down one partition  (HWDGE SBUF->SBUF copy)
  - at = sh2 + b1[0:127]                        (DVE add: sy fold)
The mask multiply (1-m)*b stays on-device (GpSimd).

Sharding: 8 cores = 4 batches x 2 channel-halves (16 ch each). Full inputs
in, full output out; host does layout/pad glue only.
"""

import numpy as np

import concourse.bass as bass
import concourse.mybir as mybir
import concourse.tile as tile
from concourse.bass_utils import run_bass_kernel_spmd

PD = 4
C = 16              # channels per core
N_CORES = 8
NV = 34             # v columns per u row (33 real + 1 zero pad)
NT = 11 * NV        # N per matmul: 11 u-rows x NV v-cols
NK = 9              # contraction tiles (127-row stride, 128-row reads)


def _split_multi_sync(nc):
    """The walrus in this env allows only ONE sync-wait per instruction.
    Hoist extra waits onto same-engine InstNoOp carriers placed just before
    the owning instruction (sequential waits on one engine == AND)."""
    ctr = 0
    for f in nc.m.functions:
        for bb in f.blocks:
            insts = list(bb.instructions)
            out = []
            changed = False
            for inst in insts:
                si = inst.sync_info
                waits = list(si.on_wait) if si and si.on_wait else []
                if len(waits) > 1:
                    for w in waits[:-1]:
                        nop = mybir.InstNoOp(name=f"waitnop-{ctr}", ins=[], outs=[])
                        ctr += 1
                        nop.engine = inst.engine
                        nop.sync_info = mybir.SyncInfo(on_wait=[w], on_update=[])
                        out.append(nop)
                    si.on_wait = [waits[-1]]
                    changed = True
                out.append(inst)
            if changed:
                bb.instructions = out
    return ctr


def _build_nc():
    f32 = mybir.dt.float32
    bf16 = mybir.dt.bfloat16
    nc = bass.Bass(enable_partition_id=False)
    # interleaved zero-padded cos pair (r-major rows F=33r+q):
    #   cdX[R, 0, 1+i, 2+j] = cosR[R-1][i,j]   (sx=0 term)
    #   cdX[R, 1, 1+i, 3+j] = cosR[R-34][i,j]  (sx=1 term: row F-33, j v-1)
    cdX = nc.declare_dram_parameter("cdX", [1144, 2, 34, 36], bf16, isOutput=False)
    # mask+bm blocks, row-duplicated per 127-stride tile: wbmD[k, p] = row 127k+p,
    # cols [0:64) = m block, [64:1088) = b block
    wbmD = nc.declare_dram_parameter("wbmD", [NK, 128, 1088], bf16, isOutput=False)
    # out: m-major [m, p, n, NT] so each m writes one DMA
    outT = nc.declare_dram_parameter("outT", [8, 128, 3, NT], bf16, isOutput=True)

    with tile.TileContext(nc) as tc:
        with (
            tc.tile_pool(name="cp", bufs=NK) as cp,
            tc.tile_pool(name="b1p", bufs=NK) as b1p,
            tc.tile_pool(name="shp", bufs=NK) as shp,
            tc.tile_pool(name="atp", bufs=NK) as atp,
            tc.tile_pool(name="wbp", bufs=3) as wbp,
            tc.tile_pool(name="msp", bufs=3) as msp,
            tc.tile_pool(name="wmp", bufs=NK) as wmp,
            tc.tile_pool(name="op", bufs=4) as op,
            tc.tile_pool(name="pp", bufs=6, space="PSUM") as pp,
        ):
            # --- input DMAs, priority-ordered on the sync queue ---
            t01s, wb3s = [None] * NK, [None] * 3
            wb3s[0] = wbp.tile([128, 3, 1088], bf16, tag="wb", name="wb3_0")
            nc.sync.dma_start(wb3s[0][:], wbmD[0:3].rearrange("k p f -> p k f"))
            for k in range(NK):
                t01 = cp.tile([128, 2, 34, 36], bf16, tag="t01", name=f"t01_{k}")
                nc.sync.dma_start(t01[:], cdX[127 * k: 127 * k + 128])
                t01s[k] = t01
                if k == 2:
                    wb3s[1] = wbp.tile([128, 3, 1088], bf16, tag="wb", name="wb3_1")
                    nc.sync.dma_start(wb3s[1][:],
                                      wbmD[3:6].rearrange("k p f -> p k f"))
                if k == 5:
                    wb3s[2] = wbp.tile([128, 3, 1088], bf16, tag="wb", name="wb3_2")
                    nc.sync.dma_start(wb3s[2][:],
                                      wbmD[6:9].rearrange("k p f -> p k f"))

            # --- A build: sx fold (DVE), sy fold (SWDGE shift-copy + DVE).
            # All DVE operands are full contiguous rows (strided windows
            # drop the DVE out of 2x mode); windows move to DMA/matmul APs.
            # HWDGE must NOT carry SBUF->SBUF traffic (measured ~25 GB/s on
            # the qAct ring); the SWDGE (gpsimd) path does it fine.
            # Vector-queue order staggers at_k two slots behind b1_k so the
            # sh2 round trip never head-of-line-blocks the queue.
            ats, b1s, sh2s, wmks = [], [], [None] * NK, [None] * NK

            def emit_b1(k):
                b1 = b1p.tile([128, 34, 36], bf16, tag="b1", name=f"b1_{k}")
                nc.vector.tensor_tensor(
                    out=b1[:], in0=t01s[k][:, 0], in1=t01s[k][:, 1],
                    op=mybir.AluOpType.add,
                )
                b1s.append(b1)
                sh2 = shp.tile([128, 33, 36], bf16, tag="sh", name=f"sh2_{k}")
                nc.gpsimd.dma_start(sh2[0:127], b1[1:128, 1:34, :])
                sh2s[k] = sh2

            def emit_at(k):
                at = atp.tile([128, 33, 36], bf16, tag="at", name=f"at_{k}")
                nc.vector.tensor_tensor(
                    out=at[:], in0=sh2s[k][:], in1=b1s[k][:, 0:33, :],
                    op=mybir.AluOpType.add,
                )
                ats.append(at)
                # mask multiply rides the vector queue right behind at_k
                g, j = k // 3, k % 3
                ms = msp.tile([128, 64], bf16, tag="ms", name=f"ms_{k}")
                nc.vector.tensor_scalar(
                    out=ms[:], in0=wb3s[g][:, j, 0:64], scalar1=-1.0, scalar2=1.0,
                    op0=mybir.AluOpType.mult, op1=mybir.AluOpType.add,
                )
                wmk = wmp.tile([128, C * 64], bf16, tag="wm", name=f"wm_{k}")
                nc.vector.tensor_tensor(
                    out=wmk[:].rearrange("p (c f) -> p c f", c=C),
                    in0=wb3s[g][:, j, 64:1088].rearrange("p (c f) -> p c f", c=C),
                    in1=ms[:, None, :].broadcast_to([128, C, 64]),
                    op=mybir.AluOpType.mult,
                )
                wmks[k] = wmk

            for k in range(NK):
                emit_b1(k)
                if k >= 2:
                    emit_at(k - 2)
            emit_at(NK - 2)
            emit_at(NK - 1)

            # --- main matmul: m-outer, k-contiguous, 3 n-phases ---
            # psum tiles padded to a full 2KB bank (no bank sharing).
            for m in range(8):
                pss = [pp.tile([128, 512], f32, tag="ps", name=f"ps_{m}_{n}")
                       for n in range(3)]
                for k in range(NK):
                    lhs = wmks[k][0:127, 128 * m: 128 * (m + 1)]
                    for n in range(3):
                        nc.tensor.matmul(
                            pss[n][:, 0:NT], lhs,
                            ats[k][0:127, 11 * n: 11 * n + 11, 2:36],
                            start=(k == 0), stop=(k == NK - 1),
                        )
                ot = op.tile([128, 3, NT], bf16, tag="o", name=f"osb_{m}")
                nc.scalar.copy(ot[:, 0, :], pss[0][:, 0:NT])
                nc.scalar.copy(ot[:, 1, :], pss[1][:, 0:NT])
                nc.scalar.copy(ot[:, 2, :], pss[2][:, 0:NT])
                nc.scalar.dma_start(outT[m], ot[:])

    _split_multi_sync(nc)
    return nc


def _host_prep(b_ch, mask_b, cos_b):
    """b_ch (16,256,256) f32, mask_b (256,256) f32, cos_b (1024,32,32) f32
    -> dict of device inputs (layout/pad glue only)."""
    import ml_dtypes
    bf16 = ml_dtypes.bfloat16
    bpad = np.pad(b_ch, ((0, 0), (PD, PD), (PD, PD)), mode="edge")
    mpad = np.pad(mask_b, ((PD, PD), (PD, PD)), mode="edge")

    cosR = np.zeros((1089, 32, 32), bf16)              # r-major rows F=33r+q
    cosR.reshape(33, 33, 32, 32)[:32, :32] = \
        cos_b.reshape(32, 32, 32, 32).transpose(1, 0, 2, 3).astype(bf16)
    cdX = np.zeros((1144, 2, 34, 36), bf16)
    cdX[1:1090, 0, 1:33, 2:34] = cosR
    cdX[34:1123, 1, 1:33, 3:35] = cosR

    wbm = np.zeros((1144, 1088), bf16)
    wbm[:1089, 0:64] = mpad.reshape(33, 8, 33, 8).transpose(2, 0, 1, 3) \
        .reshape(1089, 64).astype(bf16)
    wbm[:1089, 64:] = bpad.reshape(C, 33, 8, 33, 8).transpose(3, 1, 0, 2, 4) \
        .reshape(1089, C * 64).astype(bf16)
    wbmD = np.stack([wbm[127 * k: 127 * k + 128] for k in range(NK)])
    return {"cdX": cdX, "wbmD": wbmD}


def _unshard(outT):
    # outT [8, 128, 3, NT] -> [(c,ry,rx)=128m+p, u=11n+u', v] -> (16,256,256)
    t = np.asarray(outT, dtype=np.float32).reshape(1024, 33, NV)[:, :, :33]
    t = t.reshape(C, 8, 8, 33, 33).transpose(0, 3, 1, 4, 2)
    return t.reshape(C, 264, 264)[:, PD:260, PD:260]


_RUN_KW = {}   # test harness may inject e.g. trace=True
_LAST_RESULTS = [None]
_NC_CACHE = {}


def _get_nc():
    nc = _NC_CACHE.get("v5")
    if nc is None:
        nc = _NC_CACHE["v5"] = _build_nc()
    return nc


def kernel(cos_similar, b, mask):
    cos_similar = np.ascontiguousarray(np.asarray(cos_similar, dtype=np.float32))
    b = np.ascontiguousarray(np.asarray(b, dtype=np.float32))
    mask = np.ascontiguousarray(np.asarray(mask, dtype=np.float32))

    in_maps = []
    for core in range(N_CORES):
        batch, half = core // 2, core % 2
        ch0 = C * half
        in_maps.append(_host_prep(
            b[batch, ch0:ch0 + C], mask[batch, 0], cos_similar[batch]))

    nc = _get_nc()
    res = run_bass_kernel_spmd(nc, in_maps, list(range(N_CORES)), **_RUN_KW)
    _LAST_RESULTS[0] = res

    out = np.empty((4, 32, 256, 256), np.float32)
    for core in range(N_CORES):
        batch, half = core // 2, core % 2
        ch0 = C * half
        out[batch, ch0:ch0 + C] = _unshard(res.results[core]["outT"])
    return out


# revision 8
# speedup vs baseline: 2.6211x; 2.2951x over previous
"""Trainium2 Bass kernel for nn_CP2_17669495456475 (dynamic-kernel deconv).

Math: out[b,c,y,x] = sum_l cos[b,l,i,j] * W[b,l,c,ky,kx],  y=8i+ky, x=8j+kx,
with W = unfold(pad(b)) * (1 - unfold(pad(mask))), K=16, S=8, crop 4.

Reformulation: the 4-fold (sy,sx) deconv overlap sum is pre-folded into the
MOVING operand, so the main matmul contracts over only the 33x33=1089 bm
blocks (3.76x fewer PE cycles than the 4096-contraction chunk scheme):

  A[(q,r),(u,v)] = sum_{sy,sx in {0,1}} cos[q-sy, r-sx, u-sy, v-sx]
  out[(c,ry,rx),(u,v)] = sum_{(q,r)} bm_block[(q,r),(c,ry,rx)] * A[(q,r),(u,v)]

The four shift terms are materialized host-side as four aligned SLOTS of a
zero-padded cos tensor (pure layout duplication; the adds happen on-device),
because every on-device partition-shift path (SBUF->SBUF DMA on any DGE)
measures ~11 GB/s here. A is built slice-by-slice with two contiguous DVE
adds per slice.

Streaming order is u-phase-major: all n=0 slices, then n=1, then n=2, and
the matmul runs (n, k, m)-ordered waves with 8 single-bank PSUM groups, so
the <=8-group PSUM working set always matches the data that is arriving and
the PE consumes each slice the moment it lands. The mask multiply
(1-m)*b stays on-device (DVE).

Sharding: 8 cores = 4 batches x 2 channel-halves (16 ch each). Full inputs
in, full output out; host does layout/pad glue only.
"""

import numpy as np

import concourse.bass as bass
import concourse.mybir as mybir
import concourse.tile as tile
from concourse.bass_utils import run_bass_kernel_spmd

PD = 4
C = 16              # channels per core
N_CORES = 8
NV = 34             # v columns per u row (33 real + 1 zero pad)
NT = 11 * NV        # N per matmul: 11 u-rows x NV v-cols
NK = 9              # contraction tiles of 128 rows (q-major flat F=33q+r)


def _split_multi_sync(nc):
    """The walrus in this env allows only ONE sync-wait per instruction.
    Hoist extra waits onto same-engine InstNoOp carriers placed just before
    the owning instruction (sequential waits on one engine == AND)."""
    ctr = 0
    for f in nc.m.functions:
        for bb in f.blocks:
            insts = list(bb.instructions)
            out = []
            changed = False
            for inst in insts:
                si = inst.sync_info
                waits = list(si.on_wait) if si and si.on_wait else []
                if len(waits) > 1:
                    for w in waits[:-1]:
                        nop = mybir.InstNoOp(name=f"waitnop-{ctr}", ins=[], outs=[])
                        ctr += 1
                        nop.engine = inst.engine
                        nop.sync_info = mybir.SyncInfo(on_wait=[w], on_update=[])
                        out.append(nop)
                    si.on_wait = [waits[-1]]
                    changed = True
                out.append(inst)
            if changed:
                bb.instructions = out
    return ctr


def _build_nc():
    f32 = mybir.dt.float32
    bf16 = mybir.dt.bfloat16
    nc = bass.Bass(enable_partition_id=False)
    # four-slot zero-padded cos: slot s=(sy,sx):
    #   cdQ[33*sy+sx + F, s, 1+sy+i, 2+sx+j] = cosR[F][i,j],  cosR[33q+r]=cos4[q,r]
    cdQ = nc.declare_dram_parameter("cdQ", [1152, 4, 34, 36], bf16, isOutput=False)
    # mask+bm blocks: rows F, cols [0:64) = m block, [64:1088) = b block
    wbmD = nc.declare_dram_parameter("wbmD", [1152, 1088], bf16, isOutput=False)
    # out: m-major [m, p, n, NT] so each m writes one DMA
    outT = nc.declare_dram_parameter("outT", [8, 128, 3, NT], bf16, isOutput=True)

    with tile.TileContext(nc) as tc:
        with (
            tc.tile_pool(name="slp", bufs=6) as slp,
            tc.tile_pool(name="tmp", bufs=3) as tmp_p,
            tc.tile_pool(name="atp", bufs=27) as atp,
            tc.tile_pool(name="wbp", bufs=3) as wbp,
            tc.tile_pool(name="msp", bufs=3) as msp,
            tc.tile_pool(name="wmp", bufs=NK) as wmp,
            tc.tile_pool(name="op", bufs=8) as op,
            tc.tile_pool(name="pp", bufs=8, space="PSUM") as pp,
        ):
            wb3s = [None] * 3

            def emit_wb3(g):
                wb3s[g] = wbp.tile([128, 3, 1088], bf16, tag="wb", name=f"wb3_{g}")
                nc.sync.dma_start(
                    wb3s[g][:],
                    wbmD[384 * g: 384 * g + 384].rearrange("(k p) f -> p k f", p=128))

            slices = {}

            def emit_slice_dma(k, n):
                st = slp.tile([128, 4, 11, 36], bf16, tag="sl", name=f"sl_{k}_{n}")
                nc.sync.dma_start(
                    st[:], cdQ[128 * k: 128 * k + 128, :, 11 * n + 1: 11 * n + 12, :])
                slices[k, n] = st

            ats = {}

            def emit_slice_sum(k, n):
                st = slices[k, n]
                tm = tmp_p.tile([128, 2, 11, 36], bf16, tag="tm", name=f"tm_{k}_{n}")
                nc.vector.tensor_tensor(
                    out=tm[:], in0=st[:, 0:2], in1=st[:, 2:4],
                    op=mybir.AluOpType.add,
                )
                at = atp.tile([128, 11, 36], bf16, tag="at", name=f"at_{k}_{n}")
                nc.vector.tensor_tensor(
                    out=at[:], in0=tm[:, 0], in1=tm[:, 1],
                    op=mybir.AluOpType.add,
                )
                ats[k, n] = at

            wmks = [None] * NK

            def emit_wmask(k):
                g, j = k // 3, k % 3
                ms = msp.tile([128, 64], bf16, tag="ms", name=f"ms_{k}")
                nc.vector.tensor_scalar(
                    out=ms[:], in0=wb3s[g][:, j, 0:64], scalar1=-1.0, scalar2=1.0,
                    op0=mybir.AluOpType.mult, op1=mybir.AluOpType.add,
                )
                wmk = wmp.tile([128, C * 64], bf16, tag="wm", name=f"wm_{k}")
                nc.vector.tensor_tensor(
                    out=wmk[:].rearrange("p (c f) -> p c f", c=C),
                    in0=wb3s[g][:, j, 64:1088].rearrange("p (c f) -> p c f", c=C),
                    in1=ms[:, None, :].broadcast_to([128, C, 64]),
                    op=mybir.AluOpType.mult,
                )
                wmks[k] = wmk

            # --- DMAs: wbm first (wave-0 needs masks), then n-major slices
            emit_wb3(0)
            emit_slice_dma(0, 0)
            emit_wb3(1)
            emit_slice_dma(1, 0)
            emit_wb3(2)
            for k in range(2, NK):
                emit_slice_dma(k, 0)
            for n in (1, 2):
                for k in range(NK):
                    emit_slice_dma(k, n)

            # --- vector queue: masks + slice sums in consumption order
            emit_wmask(0)
            emit_wmask(1)
            emit_slice_sum(0, 0)
            emit_wmask(2)
            emit_slice_sum(1, 0)
            emit_wmask(3)
            emit_slice_sum(2, 0)
            emit_wmask(4)
            emit_wmask(5)
            emit_slice_sum(3, 0)
            emit_wmask(6)
            emit_slice_sum(4, 0)
            emit_wmask(7)
            emit_wmask(8)
            for k in range(5, NK):
                emit_slice_sum(k, 0)
            for n in (1, 2):
                for k in range(NK):
                    emit_slice_sum(k, n)

            # --- matmul: (n, k, m) waves; 8 single-bank psum groups per n
            osbs = [None] * 8
            for n in range(3):
                pss = [pp.tile([128, 512], f32, tag="ps", name=f"ps_{n}_{m}")
                       for m in range(8)]
                for k in range(NK):
                    rhs = ats[k, n][:, :, 2:36]
                    for m in range(8):
                        nc.tensor.matmul(
                            pss[m][:, 0:NT], wmks[k][:, 128 * m: 128 * (m + 1)],
                            rhs, start=(k == 0), stop=(k == NK - 1),
                        )
                for m in range(8):
                    if n == 0:
                        osbs[m] = op.tile([128, 3, NT], bf16, tag="o",
                                          name=f"osb_{m}")
                    nc.scalar.copy(osbs[m][:, n, :], pss[m][:, 0:NT])
                    if n == 2:
                        nc.scalar.dma_start(outT[m], osbs[m][:])

    _split_multi_sync(nc)
    return nc


def _host_prep(b_ch, mask_b, cos_b):
    """b_ch (16,256,256) f32, mask_b (256,256) f32, cos_b (1024,32,32) f32
    -> dict of device inputs (layout/pad glue only)."""
    import ml_dtypes
    bf16 = ml_dtypes.bfloat16
    bpad = np.pad(b_ch, ((0, 0), (PD, PD), (PD, PD)), mode="edge")
    mpad = np.pad(mask_b, ((PD, PD), (PD, PD)), mode="edge")

    cosR = np.zeros((1089, 32, 32), bf16)              # q-major rows F=33q+r
    cosR.reshape(33, 33, 32, 32)[:32, :32] = \
        cos_b.reshape(32, 32, 32, 32).astype(bf16)
    cdQ = np.zeros((1152, 4, 34, 36), bf16)
    for s, (sy, sx) in enumerate(((0, 0), (0, 1), (1, 0), (1, 1))):
        off = 33 * sy + sx
        cdQ[off:off + 1089, s, 1 + sy:33 + sy, 2 + sx:34 + sx] = cosR

    wbm = np.zeros((1152, 1088), bf16)
    wbm[:1089, 0:64] = mpad.reshape(33, 8, 33, 8).transpose(0, 2, 1, 3) \
        .reshape(1089, 64).astype(bf16)
    wbm[:1089, 64:] = bpad.reshape(C, 33, 8, 33, 8).transpose(1, 3, 0, 2, 4) \
        .reshape(1089, C * 64).astype(bf16)
    return {"cdQ": cdQ, "wbmD": wbm}


def _unshard(outT):
    # outT [8, 128, 3, NT] -> [(c,ry,rx)=128m+p, u=11n+u', v] -> (16,256,256)
    t = np.asarray(outT, dtype=np.float32).reshape(1024, 33, NV)[:, :, :33]
    t = t.reshape(C, 8, 8, 33, 33).transpose(0, 3, 1, 4, 2)
    return t.reshape(C, 264, 264)[:, PD:260, PD:260]


_RUN_KW = {}   # test harness may inject e.g. trace=True
_LAST_RESULTS = [None]
_NC_CACHE = {}


def _get_nc():
    nc = _NC_CACHE.get("v7")
    if nc is None:
        nc = _NC_CACHE["v7"] = _build_nc()
    return nc


def kernel(cos_similar, b, mask):
    cos_similar = np.ascontiguousarray(np.asarray(cos_similar, dtype=np.float32))
    b = np.ascontiguousarray(np.asarray(b, dtype=np.float32))
    mask = np.ascontiguousarray(np.asarray(mask, dtype=np.float32))

    in_maps = []
    for core in range(N_CORES):
        batch, half = core // 2, core % 2
        ch0 = C * half
        in_maps.append(_host_prep(
            b[batch, ch0:ch0 + C], mask[batch, 0], cos_similar[batch]))

    nc = _get_nc()
    res = run_bass_kernel_spmd(nc, in_maps, list(range(N_CORES)), **_RUN_KW)
    _LAST_RESULTS[0] = res

    out = np.empty((4, 32, 256, 256), np.float32)
    for core in range(N_CORES):
        batch, half = core // 2, core % 2
        ch0 = C * half
        out[batch, ch0:ch0 + C] = _unshard(res.results[core]["outT"])
    return out


# revision 12
# speedup vs baseline: 2.7056x; 1.0322x over previous
"""Trainium2 Bass kernel for nn_CP2_17669495456475 (dynamic-kernel deconv).

Math: out[b,c,y,x] = sum_l cos[b,l,i,j] * W[b,l,c,ky,kx],  y=8i+ky, x=8j+kx,
with W = unfold(pad(b)) * (1 - unfold(pad(mask))), K=16, S=8, crop 4.

Reformulation: the 4-fold (sy,sx) deconv overlap sum is pre-folded into the
MOVING operand, so the main matmul contracts over only the 33x33=1089 bm
blocks (3.76x fewer PE cycles than the 4096-contraction chunk scheme):

  A[(q,r),(u,v)] = sum_{sy,sx in {0,1}} cos[q-sy, r-sx, u-sy, v-sx]
  out[(c,ry,rx),(u,v)] = sum_{(q,r)} bm_block[(q,r),(c,ry,rx)] * A[(q,r),(u,v)]

The four shift terms are materialized host-side as four aligned SLOTS of a
zero-padded cos tensor (pure layout duplication; the adds happen on-device),
because every on-device partition-shift path (SBUF->SBUF DMA on any DGE)
measures ~11 GB/s here. A is built slice-by-slice with two contiguous DVE
adds per slice.

Streaming order is u-phase-major: all n=0 slices, then n=1, then n=2, and
the matmul runs (n, k, m)-ordered waves with 8 single-bank PSUM groups, so
the <=8-group PSUM working set always matches the data that is arriving and
the PE consumes each slice the moment it lands. The mask multiply
(1-m)*b stays on-device (DVE).

Sharding: 8 cores = 4 batches x 2 channel-halves (16 ch each). Full inputs
in, full output out; host does layout/pad glue only.
"""

import numpy as np

import concourse.bass as bass
import concourse.mybir as mybir
import concourse.tile as tile
from concourse.bass_utils import run_bass_kernel_spmd

PD = 4
C = 16              # channels per core
N_CORES = 8
NV = 34             # v columns per u row (33 real + 1 zero pad)
NT = 11 * NV        # N per matmul: 11 u-rows x NV v-cols
NK = 9              # contraction tiles of 128 rows (q-major flat F=33q+r)


def _split_multi_sync(nc):
    """The walrus in this env allows only ONE sync-wait per instruction.
    Hoist extra waits onto same-engine InstNoOp carriers placed just before
    the owning instruction (sequential waits on one engine == AND)."""
    ctr = 0
    for f in nc.m.functions:
        for bb in f.blocks:
            insts = list(bb.instructions)
            out = []
            changed = False
            for inst in insts:
                si = inst.sync_info
                waits = list(si.on_wait) if si and si.on_wait else []
                if len(waits) > 1:
                    for w in waits[:-1]:
                        nop = mybir.InstNoOp(name=f"waitnop-{ctr}", ins=[], outs=[])
                        ctr += 1
                        nop.engine = inst.engine
                        nop.sync_info = mybir.SyncInfo(on_wait=[w], on_update=[])
                        out.append(nop)
                    si.on_wait = [waits[-1]]
                    changed = True
                out.append(inst)
            if changed:
                bb.instructions = out
    return ctr


def _build_nc():
    f32 = mybir.dt.float32
    bf16 = mybir.dt.bfloat16
    nc = bass.Bass(enable_partition_id=False)
    # four-slot zero-padded cos: slot s=(sy,sx):
    #   cdQ[33*sy+sx + F, s, 1+sy+i, 2+sx+j] = cosR[F][i,j],  cosR[33q+r]=cos4[q,r]
    cdQ = nc.declare_dram_parameter("cdQ", [1152, 4, 34, 36], bf16, isOutput=False)
    # mask+bm blocks: rows F, cols [0:64) = m block, [64:1088) = b block
    wbmD = nc.declare_dram_parameter("wbmD", [1152, 1088], bf16, isOutput=False)
    # out: m-major [m, p, n, NT] so each m writes one DMA
    outT = nc.declare_dram_parameter("outT", [8, 128, 3, NT], bf16, isOutput=True)

    with tile.TileContext(nc) as tc:
        with (
            tc.tile_pool(name="slp", bufs=6) as slp,
            tc.tile_pool(name="tmp", bufs=3) as tmp_p,
            tc.tile_pool(name="atp", bufs=27) as atp,
            tc.tile_pool(name="wbp", bufs=NK) as wbp,
            tc.tile_pool(name="msp", bufs=3) as msp,
            tc.tile_pool(name="wmp", bufs=NK) as wmp,
            tc.tile_pool(name="op", bufs=8) as op,
            tc.tile_pool(name="pp", bufs=8, space="PSUM") as pp,
        ):
            wbts = [None] * NK

            def emit_wbt(k):
                wbts[k] = wbp.tile([128, 1088], bf16, tag="wb", name=f"wbt_{k}")
                nc.sync.dma_start(wbts[k][:], wbmD[128 * k: 128 * k + 128])

            slices = {}

            def emit_slice_dma(k, n):
                st = slp.tile([128, 4, 11, 36], bf16, tag="sl", name=f"sl_{k}_{n}")
                nc.sync.dma_start(
                    st[:], cdQ[128 * k: 128 * k + 128, :, 11 * n + 1: 11 * n + 12, :])
                slices[k, n] = st

            ats = {}

            def emit_slice_sum(k, n):
                st = slices[k, n]
                tm = tmp_p.tile([128, 2, 11, 36], bf16, tag="tm", name=f"tm_{k}_{n}")
                nc.vector.tensor_tensor(
                    out=tm[:], in0=st[:, 0:2], in1=st[:, 2:4],
                    op=mybir.AluOpType.add,
                )
                at = atp.tile([128, 11, 36], bf16, tag="at", name=f"at_{k}_{n}")
                nc.vector.tensor_tensor(
                    out=at[:], in0=tm[:, 0], in1=tm[:, 1],
                    op=mybir.AluOpType.add,
                )
                ats[k, n] = at

            wmks = [None] * NK

            def emit_wmask(k):
                ms = msp.tile([128, 64], bf16, tag="ms", name=f"ms_{k}")
                nc.vector.tensor_scalar(
                    out=ms[:], in0=wbts[k][:, 0:64], scalar1=-1.0, scalar2=1.0,
                    op0=mybir.AluOpType.mult, op1=mybir.AluOpType.add,
                )
                wmk = wmp.tile([128, C * 64], bf16, tag="wm", name=f"wm_{k}")
                nc.vector.tensor_tensor(
                    out=wmk[:].rearrange("p (c f) -> p c f", c=C),
                    in0=wbts[k][:, 64:1088].rearrange("p (c f) -> p c f", c=C),
                    in1=ms[:, None, :].broadcast_to([128, C, 64]),
                    op=mybir.AluOpType.mult,
                )
                wmks[k] = wmk

            # --- DMAs: wbm_0 + first slice lead; then 1:1 interleave
            emit_wbt(0)
            emit_slice_dma(0, 0)
            for k in range(1, NK):
                emit_wbt(k)
                emit_slice_dma(k, 0)
            for n in (1, 2):
                for k in range(NK):
                    emit_slice_dma(k, n)

            # --- PE warmup: ~4us of garbage matmuls gated on the first wbm
            # read, so the HAM clock gate is already at K=8/8 (2.4 GHz) when
            # the real stream starts (saves ~6us of 1.2 GHz matmuls). They
            # target group (0,0)'s bank; its real start-matmul clears it.
            pss0 = [pp.tile([128, 512], f32, tag="ps", name=f"ps_0_{m}")
                    for m in range(8)]
            for i in range(18):
                nc.tensor.matmul(pss0[0][:, 0:256], wbts[0][:, 0:128],
                                 wbts[0][:, 0:256], start=True, stop=True)

            # --- vector queue: masks + slice sums in consumption order
            emit_wmask(0)
            emit_slice_sum(0, 0)
            for k in range(1, NK):
                emit_wmask(k)
                emit_slice_sum(k, 0)
            for n in (1, 2):
                for k in range(NK):
                    emit_slice_sum(k, n)

            # --- matmul: (n, k, m) waves; 8 single-bank psum groups per n.
            # The stop wave interleaves each m's evacuation + store right
            # behind its stop matmul, split across engines/queues, so the
            # post-stream tail is one evac+store instead of eight.
            osbs = [None] * 8
            for n in range(3):
                pss = pss0 if n == 0 else \
                    [pp.tile([128, 512], f32, tag="ps", name=f"ps_{n}_{m}")
                     for m in range(8)]
                for k in range(NK):
                    rhs = ats[k, n][:, :, 2:36]
                    for m in range(8):
                        nc.tensor.matmul(
                            pss[m][:, 0:NT], wmks[k][:, 128 * m: 128 * (m + 1)],
                            rhs, start=(k == 0), stop=(k == NK - 1),
                        )
                        if k == NK - 1:
                            if n == 0:
                                osbs[m] = op.tile([128, 3, NT], bf16, tag="o",
                                                  name=f"osb_{m}")
                            if m % 2 == 0:
                                nc.scalar.copy(osbs[m][:, n, :], pss[m][:, 0:NT])
                            else:
                                nc.vector.tensor_copy(osbs[m][:, n, :],
                                                      pss[m][:, 0:NT])
                            if n == 2:
                                eng = nc.scalar if m % 2 == 0 else nc.sync
                                eng.dma_start(outT[m], osbs[m][:])

    _split_multi_sync(nc)
    return nc


def _host_prep(b_ch, mask_b, cos_b):
    """b_ch (16,256,256) f32, mask_b (256,256) f32, cos_b (1024,32,32) f32
    -> dict of device inputs (layout/pad glue only)."""
    import ml_dtypes
    bf16 = ml_dtypes.bfloat16
    bpad = np.pad(b_ch, ((0, 0), (PD, PD), (PD, PD)), mode="edge")
    mpad = np.pad(mask_b, ((PD, PD), (PD, PD)), mode="edge")

    cosR = np.zeros((1089, 32, 32), bf16)              # q-major rows F=33q+r
    cosR.reshape(33, 33, 32, 32)[:32, :32] = \
        cos_b.reshape(32, 32, 32, 32).astype(bf16)
    cdQ = np.zeros((1152, 4, 34, 36), bf16)
    for s, (sy, sx) in enumerate(((0, 0), (0, 1), (1, 0), (1, 1))):
        off = 33 * sy + sx
        cdQ[off:off + 1089, s, 1 + sy:33 + sy, 2 + sx:34 + sx] = cosR

    wbm = np.zeros((1152, 1088), bf16)
    wbm[:1089, 0:64] = mpad.reshape(33, 8, 33, 8).transpose(0, 2, 1, 3) \
        .reshape(1089, 64).astype(bf16)
    wbm[:1089, 64:] = bpad.reshape(C, 33, 8, 33, 8).transpose(1, 3, 0, 2, 4) \
        .reshape(1089, C * 64).astype(bf16)
    return {"cdQ": cdQ, "wbmD": wbm}


def _unshard(outT):
    # outT [8, 128, 3, NT] -> [(c,ry,rx)=128m+p, u=11n+u', v] -> (16,256,256)
    t = np.asarray(outT, dtype=np.float32).reshape(1024, 33, NV)[:, :, :33]
    t = t.reshape(C, 8, 8, 33, 33).transpose(0, 3, 1, 4, 2)
    return t.reshape(C, 264, 264)[:, PD:260, PD:260]


_RUN_KW = {}   # test harness may inject e.g. trace=True
_LAST_RESULTS = [None]
_NC_CACHE = {}


def _get_nc():
    nc = _NC_CACHE.get("v7")
    if nc is None:
        nc = _NC_CACHE["v7"] = _build_nc()
    return nc


def kernel(cos_similar, b, mask):
    cos_similar = np.ascontiguousarray(np.asarray(cos_similar, dtype=np.float32))
    b = np.ascontiguousarray(np.asarray(b, dtype=np.float32))
    mask = np.ascontiguousarray(np.asarray(mask, dtype=np.float32))

    in_maps = []
    for core in range(N_CORES):
        batch, half = core // 2, core % 2
        ch0 = C * half
        in_maps.append(_host_prep(
            b[batch, ch0:ch0 + C], mask[batch, 0], cos_similar[batch]))

    nc = _get_nc()
    res = run_bass_kernel_spmd(nc, in_maps, list(range(N_CORES)), **_RUN_KW)
    _LAST_RESULTS[0] = res

    out = np.empty((4, 32, 256, 256), np.float32)
    for core in range(N_CORES):
        batch, half = core // 2, core % 2
        ch0 = C * half
        out[batch, ch0:ch0 + C] = _unshard(res.results[core]["outT"])
    return out
